# revision 22
# baseline (speedup 1.0000x reference)
"""Trainium2 Bass kernel: causal single-head attention with attention-prob
dropout (train mode, fixed threefry key), data-parallel over batch on 8 cores.

Shapes (hardcoded): x [2048,128,256] f32, Wq/Wk/Wv [256,64] f32.
out [2048,128,64] f32.

Per-core design (256 batches/core):
  - host pre-packs x TRANSPOSED and bf16: xtp[e, b*T + t]; group DMA loads
    are contiguous per partition, and no on-device transpose is needed.
  - qkT = [Wq/8 | Wk].T @ x^T  -> [128, 128] psum (rows 0-63 qT, 64-127 kT)
  - v   = x @ Wv               -> [128, 64]
  - S^T in [s, t]: psum preloaded with causal -1e30 mask via an
    identity-stationary PE matmul, then += kT.T @ qT.
  - exp on ACT straight from PSUM (bf16 out); denominator d[t] via
    ones-matmul over s (partition reduction on PE); reciprocal on DVE.
  - numerator: PD = exp * dropmask({0,1.25}) (bf16), out = PD.T @ v,
    final scale by r[t] into the [t, h] output tile.
  - QUAD batching: psum tiles hold 4 batches per bank; exp / copies /
    dropout-mul are issued as single [128, 512]-ish ops over 4 batches.
"""

import os
import sys
import numpy as np

sys.path.insert(0, "/opt/trn_rl_repo")

import ml_dtypes

BF16_NP = ml_dtypes.bfloat16

B, T, E, H = 2048, 128, 256, 64
N_CORES = 8
B_PER_CORE = B // N_CORES
DROP_P = 0.2

_CACHE = {}


def _build_nc(n_batches, g=32):
    import concourse.bass as bass
    import concourse.mybir as mybir
    from concourse import bacc, tile
    from contextlib import ExitStack

    BF16 = mybir.dt.bfloat16
    F32 = mybir.dt.float32

    nc = bacc.Bacc()
    xt_p = nc.declare_dram_parameter("xtp", [E, n_batches * T], BF16,
                                     isOutput=False)
    FP8 = mybir.dt.float8e4
    dt_p = nc.declare_dram_parameter("dtp", [T, n_batches * T], FP8,
                                     isOutput=False)
    wqk_p = nc.declare_dram_parameter("wqk", [E, 128], BF16, isOutput=False)
    wv_p = nc.declare_dram_parameter("wv", [E, H], BF16, isOutput=False)
    cneg_p = nc.declare_dram_parameter("cneg", [T, 4 * T], BF16,
                                       isOutput=False)
    id_p = nc.declare_dram_parameter("ident", [T, T], BF16, isOutput=False)
    out_p = nc.declare_dram_parameter("out", [T, n_batches * H], F32,
                                      isOutput=True)

    n_groups = n_batches // g
    QUAD = 4
    n_quads = g // QUAD

    with tile.TileContext(nc) as tc, ExitStack() as ctx:
        const = ctx.enter_context(tc.tile_pool(name="const", bufs=1))
        wqk_sb = [const.tile([128, 128], BF16, tag=f"wqk{c}", name=f"wqk{c}")
                  for c in range(2)]
        wv_sb = [const.tile([128, H], BF16, tag=f"wv{c}", name=f"wv{c}")
                 for c in range(2)]
        cneg_sb = const.tile([T, 4 * T], BF16, tag="cneg", name="cneg")
        ident_sb = const.tile([T, T], BF16, tag="ident", name="ident")
        ones_sb = const.tile([128, 1], BF16, tag="ones", name="ones")
        for c in range(2):
            nc.sync.dma_start(out=wqk_sb[c][:],
                              in_=wqk_p[c * 128:(c + 1) * 128, :])
            nc.sync.dma_start(out=wv_sb[c][:],
                              in_=wv_p[c * 128:(c + 1) * 128, :])
        nc.sync.dma_start(out=cneg_sb[:], in_=cneg_p[:])
        nc.sync.dma_start(out=ident_sb[:], in_=id_p[:])
        nc.vector.memset(ones_sb[:], 1.0)

        xpool = ctx.enter_context(tc.tile_pool(name="xtg", bufs=2))
        dtpool = ctx.enter_context(tc.tile_pool(name="dtg", bufs=2))
        opool = ctx.enter_context(tc.tile_pool(name="og", bufs=2))
        work = ctx.enter_context(tc.tile_pool(name="work", bufs=6))
        ps_qk = ctx.enter_context(tc.tile_pool(name="ps_qk", bufs=1,
                                               space="PSUM"))
        ps_s = ctx.enter_context(tc.tile_pool(name="ps_s", bufs=3,
                                              space="PSUM"))
        ps_v = ctx.enter_context(tc.tile_pool(name="ps_v", bufs=2,
                                              space="PSUM"))
        ps_od = ctx.enter_context(tc.tile_pool(name="ps_od", bufs=2,
                                               space="PSUM"))

        EXP = mybir.ActivationFunctionType.Exp

        for grp in range(n_groups):
            g0 = grp * g
            xtg = [xpool.tile([128, g * T], BF16, tag=f"xtg{c}",
                              name=f"xtg{c}") for c in range(2)]
            dma_engs = [nc.sync, nc.scalar]
            for c in range(2):
                dma_engs[c].dma_start(
                    out=xtg[c][:],
                    in_=xt_p[c * 128:(c + 1) * 128, g0 * T:(g0 + g) * T])
            dtg = dtpool.tile([T, g * T], BF16, tag="dtg", name="dtg")
            nc.gpsimd.dma_start(out=dtg[:],
                                in_=dt_p[:, g0 * T:(g0 + g) * T])
            og = opool.tile([T, g * H], F32, tag="og", name="og")
            for q in range(n_quads):
                j0 = q * QUAD      # first batch (within group) of this quad

                # ---- PE: causal-mask preload into the S quad bank ----
                # ONE matmul covers the bank: start=True clears the whole
                # bank's has_written bits, so per-slice preloads would wipe
                # each other.
                p_s = ps_s.tile([128, QUAD * T], F32, tag="p_s", name="p_s")
                nc.tensor.matmul(p_s[:], ident_sb[:], cneg_sb[:],
                                 start=True, stop=False)

                # ---- PE: qkT for 4 batches, one weight load per chunk ----
                # only the first matmul touching the bank clears it
                p_qk = ps_qk.tile([128, QUAD * T], F32, tag="p_qk",
                                  name="p_qk")
                qsl = slice(j0 * T, (j0 + QUAD) * T)
                for c in range(2):
                    nc.tensor.matmul(p_qk[:], wqk_sb[c][:], xtg[c][:, qsl],
                                     start=(c == 0), stop=(c == 1))
                qt_sb = work.tile([64, QUAD * T], BF16, tag="qt_sb",
                                  name="qt_sb")
                nc.vector.tensor_copy(qt_sb[:], p_qk[0:64, :])
                # kT at base partition 0 (matmul requires matching bases);
                # ACT sits close to PSUM and has slack
                kt_sb = work.tile([64, QUAD * T], BF16, tag="kt_sb",
                                  name="kt_sb")
                nc.scalar.copy(kt_sb[:], p_qk[64:128, :])

                # ---- PE: v for 4 batches ----
                p_v = ps_v.tile([128, QUAD * H], F32, tag="p_v", name="p_v")
                for j in range(QUAD):
                    sl = slice((j0 + j) * T, (j0 + j + 1) * T)
                    for c in range(2):
                        nc.tensor.matmul(p_v[:, j * H:(j + 1) * H],
                                         xtg[c][:, sl], wv_sb[c][:],
                                         start=(c == 0 and j == 0),
                                         stop=(c == 1))
                v_sb = work.tile([128, QUAD * H], BF16, tag="v_sb",
                                 name="v_sb")
                nc.vector.tensor_copy(v_sb[:], p_v[:])

                # ---- PE: S^T += kT.T @ qT (accumulates onto mask) ----
                for j in range(QUAD):
                    nc.tensor.matmul(p_s[:, j * T:(j + 1) * T],
                                     kt_sb[:, j * T:(j + 1) * T],
                                     qt_sb[:, j * T:(j + 1) * T],
                                     start=False, stop=True)

                # ---- ACT: exp over the whole quad, psum -> sbuf bf16 ----
                e_sb = work.tile([128, QUAD * T], BF16, tag="e_sb",
                                 name="e_sb")
                nc.scalar.activation(e_sb[:], p_s[:], EXP)

                # ---- PE: denominators; DVE: reciprocal ----
                p_od = ps_od.tile([128, QUAD * (H + 1)], F32, tag="p_od",
                                  name="p_od")
                d_off = QUAD * H
                for j in range(QUAD):
                    nc.tensor.matmul(p_od[:, d_off + j:d_off + j + 1],
                                     e_sb[:, j * T:(j + 1) * T], ones_sb[:],
                                     start=(j == 0), stop=True)
                r_sb = work.tile([128, QUAD], F32, tag="r_sb", name="r_sb")
                nc.vector.reciprocal(r_sb[:], p_od[:, d_off:d_off + QUAD])

                # ---- DVE: dropout multiply over the quad ----
                pdm = work.tile([128, QUAD * T], BF16, tag="pdm", name="pdm")
                half = QUAD * T // 2
                nc.vector.tensor_mul(
                    pdm[:, 0:half], e_sb[:, 0:half],
                    dtg[:, j0 * T:j0 * T + half])
                nc.gpsimd.tensor_mul(
                    pdm[:, half:], e_sb[:, half:],
                    dtg[:, j0 * T + half:(j0 + QUAD) * T])

                # ---- PE: out numerator; DVE: per-batch r scale ----
                # start=False: the denominator slices live in the same bank
                # and must not be wiped; fresh elements overwrite via the
                # per-element has_written bits
                for j in range(QUAD):
                    nc.tensor.matmul(p_od[:, j * H:(j + 1) * H],
                                     pdm[:, j * T:(j + 1) * T],
                                     v_sb[:, j * H:(j + 1) * H],
                                     start=False, stop=True)
                nc.vector.tensor_mul(
                    og[:, j0 * H:(j0 + QUAD) * H].rearrange(
                        "p (f g) -> p f g", f=QUAD),
                    p_od[:, 0:QUAD * H].rearrange("p (f g) -> p f g", f=QUAD),
                    r_sb[:].broadcast_to([128, QUAD, H]))
            nc.sync.dma_start(out=out_p[:, g0 * H:(g0 + g) * H],
                              in_=og[:])
    nc.compile()
    return nc


def _get_nc(n_batches, g=32):
    key = (n_batches, g)
    if key not in _CACHE:
        _CACHE[key] = _build_nc(n_batches, g)
    return _CACHE[key]


def _dropout_keep_mask():
    """Reproduce the reference's threefry draw bit-exactly (CPU backend)."""
    if "keep" not in _CACHE:
        import jax
        cpu = jax.devices("cpu")[0]
        with jax.default_device(cpu):
            keep = jax.random.bernoulli(jax.random.key(42), 1.0 - DROP_P,
                                        (B, T, T))
            _CACHE["keep"] = np.asarray(keep)
    return _CACHE["keep"]


def _static_inputs(Wq, Wk, Wv):
    wqk = np.concatenate([Wq * 0.125, Wk], axis=1).astype(BF16_NP)
    wv = Wv.astype(BF16_NP)
    s_idx = np.arange(T)
    cneg1 = np.where(s_idx[:, None] <= s_idx[None, :], 0.0, -1e30).astype(
        BF16_NP)
    cneg = np.tile(cneg1, (1, 4))
    ident = np.eye(T, dtype=BF16_NP)
    return wqk, wv, cneg, ident


def _run(x, Wq, Wk, Wv, trace=False):
    from concourse.bass_utils import run_bass_kernel_spmd

    x = np.asarray(x, dtype=np.float32)
    Wq = np.asarray(Wq, dtype=np.float32)
    Wk = np.asarray(Wk, dtype=np.float32)
    Wv = np.asarray(Wv, dtype=np.float32)

    nc = _get_nc(B_PER_CORE)
    wqk, wv, cneg, ident = _static_inputs(Wq, Wk, Wv)

    keep = _dropout_keep_mask()
    import ml_dtypes as _mld
    dmask = np.where(keep, np.float32(1.25), np.float32(0.0)).astype(
        _mld.float8_e4m3)

    in_maps = []
    for i in range(N_CORES):
        sl = slice(i * B_PER_CORE, (i + 1) * B_PER_CORE)
        # x^T packed: [e, (b t)], bf16 (same rounding the DMA cast applied)
        xtp = np.ascontiguousarray(
            x[sl].transpose(2, 0, 1)).reshape(E, B_PER_CORE * T).astype(
                BF16_NP)
        # dropmask^T per batch: [s, (b t)]
        dtp = np.ascontiguousarray(
            dmask[sl].transpose(2, 0, 1)).reshape(T, B_PER_CORE * T)
        in_maps.append({
            "xtp": xtp, "dtp": dtp, "wqk": wqk, "wv": wv,
            "cneg": cneg, "ident": ident,
        })

    res = run_bass_kernel_spmd(nc, in_maps, list(range(N_CORES)), trace=trace)
    outs = []
    for i in range(N_CORES):
        o = res.results[i]["out"]  # [t, (b h)]
        outs.append(o.reshape(T, B_PER_CORE, H).transpose(1, 0, 2))
    full = np.ascontiguousarray(np.concatenate(outs, axis=0), dtype=np.float32)
    return full, res


def kernel(x, Wq, Wk, Wv):
    return _run(x, Wq, Wk, Wv, trace=False)[0]


# revision 23
# speedup vs baseline: 1.2597x; 1.2597x over previous
"""Trainium2 Bass kernel: causal single-head attention with attention-prob
dropout (train mode, fixed threefry key), data-parallel over batch on 8 cores.

Shapes (hardcoded): x [2048,128,256] f32, Wq/Wk/Wv [256,64] f32.
out [2048,128,64] f32.

Per-core design (256 batches/core):
  - host pre-packs x TRANSPOSED and bf16: xtp[e, b*T + t]; group DMA loads
    are contiguous per partition, and no on-device transpose is needed.
  - qkT = [Wq/8 | Wk].T @ x^T  -> [128, 128] psum (rows 0-63 qT, 64-127 kT)
  - v   = x @ Wv               -> [128, 64]
  - S^T in [s, t]: psum preloaded with causal -1e30 mask via an
    identity-stationary PE matmul, then += kT.T @ qT.
  - exp on ACT straight from PSUM (bf16 out); denominator d[t] via
    ones-matmul over s (partition reduction on PE); reciprocal on DVE.
  - numerator: PD = exp * dropmask({0,1.25}) (bf16), out = PD.T @ v,
    final scale by r[t] into the [t, h] output tile.
  - QUAD batching: psum tiles hold 4 batches per bank; exp / copies /
    dropout-mul are issued as single [128, 512]-ish ops over 4 batches.
"""

import os
import sys
import numpy as np

sys.path.insert(0, "/opt/trn_rl_repo")

import ml_dtypes

BF16_NP = ml_dtypes.bfloat16

B, T, E, H = 2048, 128, 256, 64
N_CORES = 8
B_PER_CORE = B // N_CORES
DROP_P = 0.2

_CACHE = {}


def _build_nc(n_batches, g=32):
    import concourse.bass as bass
    import concourse.mybir as mybir
    from concourse import bacc, tile
    from contextlib import ExitStack

    BF16 = mybir.dt.bfloat16
    F32 = mybir.dt.float32

    nc = bacc.Bacc()
    xt_p = nc.declare_dram_parameter("xtp", [E, n_batches * T], BF16,
                                     isOutput=False)
    FP8 = mybir.dt.float8e4
    dt_p = nc.declare_dram_parameter("dtp", [T, n_batches * T], FP8,
                                     isOutput=False)
    wqk_p = nc.declare_dram_parameter("wqk", [E, 128], BF16, isOutput=False)
    wv_p = nc.declare_dram_parameter("wv", [E, H], BF16, isOutput=False)
    cneg_p = nc.declare_dram_parameter("cneg", [T, 4 * T], BF16,
                                       isOutput=False)
    id_p = nc.declare_dram_parameter("ident", [T, T], BF16, isOutput=False)
    out_p = nc.declare_dram_parameter("out", [T, n_batches * H], F32,
                                      isOutput=True)

    n_groups = n_batches // g
    QUAD = 4
    n_quads = g // QUAD

    with tile.TileContext(nc) as tc, ExitStack() as ctx:
        const = ctx.enter_context(tc.tile_pool(name="const", bufs=1))
        wqk_sb = [const.tile([128, 128], BF16, tag=f"wqk{c}", name=f"wqk{c}")
                  for c in range(2)]
        wv_sb = [const.tile([128, H], BF16, tag=f"wv{c}", name=f"wv{c}")
                 for c in range(2)]
        cneg_sb = const.tile([T, 4 * T], BF16, tag="cneg", name="cneg")
        ident_sb = const.tile([T, T], BF16, tag="ident", name="ident")
        ones_sb = const.tile([128, 1], BF16, tag="ones", name="ones")
        for c in range(2):
            nc.sync.dma_start(out=wqk_sb[c][:],
                              in_=wqk_p[c * 128:(c + 1) * 128, :])
            nc.sync.dma_start(out=wv_sb[c][:],
                              in_=wv_p[c * 128:(c + 1) * 128, :])
        nc.sync.dma_start(out=cneg_sb[:], in_=cneg_p[:])
        nc.sync.dma_start(out=ident_sb[:], in_=id_p[:])
        nc.vector.memset(ones_sb[:], 1.0)

        xpool = ctx.enter_context(tc.tile_pool(name="xtg", bufs=2))
        dtpool = ctx.enter_context(tc.tile_pool(name="dtg", bufs=2))
        opool = ctx.enter_context(tc.tile_pool(name="og", bufs=2))
        work = ctx.enter_context(tc.tile_pool(name="work", bufs=6))
        ps_qk = ctx.enter_context(tc.tile_pool(name="ps_qk", bufs=2,
                                               space="PSUM"))
        ps_s = ctx.enter_context(tc.tile_pool(name="ps_s", bufs=2,
                                              space="PSUM"))
        ps_v = ctx.enter_context(tc.tile_pool(name="ps_v", bufs=2,
                                              space="PSUM"))
        ps_od = ctx.enter_context(tc.tile_pool(name="ps_od", bufs=2,
                                               space="PSUM"))

        EXP = mybir.ActivationFunctionType.Exp

        for grp in range(n_groups):
            g0 = grp * g
            xtg = [xpool.tile([128, g * T], BF16, tag=f"xtg{c}",
                              name=f"xtg{c}") for c in range(2)]
            dma_engs = [nc.sync, nc.scalar]
            for c in range(2):
                dma_engs[c].dma_start(
                    out=xtg[c][:],
                    in_=xt_p[c * 128:(c + 1) * 128, g0 * T:(g0 + g) * T])
            dtg = dtpool.tile([T, g * T], BF16, tag="dtg", name="dtg")
            nc.gpsimd.dma_start(out=dtg[:],
                                in_=dt_p[:, g0 * T:(g0 + g) * T])
            og = opool.tile([T, g * H], F32, tag="og", name="og")
            for q in range(n_quads):
                j0 = q * QUAD      # first batch (within group) of this quad

                # ---- PE: causal-mask preload into the S quad bank ----
                # ONE matmul covers the bank: start=True clears the whole
                # bank's has_written bits, so per-slice preloads would wipe
                # each other.
                p_s = ps_s.tile([128, QUAD * T], F32, tag="p_s", name="p_s")
                nc.tensor.matmul(p_s[:], ident_sb[:], cneg_sb[:],
                                 start=True, stop=False)

                # ---- PE: qkT for 4 batches, one weight load per chunk ----
                # only the first matmul touching the bank clears it
                p_qk = ps_qk.tile([128, QUAD * T], F32, tag="p_qk",
                                  name="p_qk")
                qsl = slice(j0 * T, (j0 + QUAD) * T)
                for c in range(2):
                    nc.tensor.matmul(p_qk[:], wqk_sb[c][:], xtg[c][:, qsl],
                                     start=(c == 0), stop=(c == 1))
                qt_sb = work.tile([64, QUAD * T], BF16, tag="qt_sb",
                                  name="qt_sb")
                nc.vector.tensor_copy(qt_sb[:], p_qk[0:64, :])
                # kT at base partition 0 (matmul requires matching bases);
                # ACT sits close to PSUM and has slack
                kt_sb = work.tile([64, QUAD * T], BF16, tag="kt_sb",
                                  name="kt_sb")
                nc.scalar.copy(kt_sb[:], p_qk[64:128, :])

                # ---- PE: v for 4 batches ----
                p_v = ps_v.tile([128, QUAD * H], F32, tag="p_v", name="p_v")
                for j in range(QUAD):
                    sl = slice((j0 + j) * T, (j0 + j + 1) * T)
                    for c in range(2):
                        nc.tensor.matmul(p_v[:, j * H:(j + 1) * H],
                                         xtg[c][:, sl], wv_sb[c][:],
                                         start=(c == 0 and j == 0),
                                         stop=(c == 1))
                v_sb = work.tile([128, QUAD * H], BF16, tag="v_sb",
                                 name="v_sb")
                nc.vector.tensor_copy(v_sb[:], p_v[:])

                # ---- PE: S^T += kT.T @ qT (accumulates onto mask) ----
                for j in range(QUAD):
                    nc.tensor.matmul(p_s[:, j * T:(j + 1) * T],
                                     kt_sb[:, j * T:(j + 1) * T],
                                     qt_sb[:, j * T:(j + 1) * T],
                                     start=False, stop=True)

                # ---- ACT: exp over the whole quad, psum -> sbuf bf16 ----
                e_sb = work.tile([128, QUAD * T], BF16, tag="e_sb",
                                 name="e_sb")
                nc.scalar.activation(e_sb[:], p_s[:], EXP)

                # ---- PE: denominators; DVE: reciprocal ----
                p_od = ps_od.tile([128, QUAD * (H + 1)], F32, tag="p_od",
                                  name="p_od")
                d_off = QUAD * H
                for j in range(QUAD):
                    nc.tensor.matmul(p_od[:, d_off + j:d_off + j + 1],
                                     e_sb[:, j * T:(j + 1) * T], ones_sb[:],
                                     start=(j == 0), stop=True)
                r_sb = work.tile([128, QUAD], F32, tag="r_sb", name="r_sb")
                nc.vector.reciprocal(r_sb[:], p_od[:, d_off:d_off + QUAD])

                # ---- DVE: dropout multiply over the quad ----
                pdm = work.tile([128, QUAD * T], BF16, tag="pdm", name="pdm")
                half = QUAD * T // 2
                nc.vector.tensor_mul(
                    pdm[:, 0:half], e_sb[:, 0:half],
                    dtg[:, j0 * T:j0 * T + half])
                nc.gpsimd.tensor_mul(
                    pdm[:, half:], e_sb[:, half:],
                    dtg[:, j0 * T + half:(j0 + QUAD) * T])

                # ---- PE: out numerator; DVE: per-batch r scale ----
                # start=False: the denominator slices live in the same bank
                # and must not be wiped; fresh elements overwrite via the
                # per-element has_written bits
                for j in range(QUAD):
                    nc.tensor.matmul(p_od[:, j * H:(j + 1) * H],
                                     pdm[:, j * T:(j + 1) * T],
                                     v_sb[:, j * H:(j + 1) * H],
                                     start=False, stop=True)
                nc.vector.tensor_mul(
                    og[:, j0 * H:(j0 + QUAD) * H].rearrange(
                        "p (f g) -> p f g", f=QUAD),
                    p_od[:, 0:QUAD * H].rearrange("p (f g) -> p f g", f=QUAD),
                    r_sb[:].broadcast_to([128, QUAD, H]))
            nc.sync.dma_start(out=out_p[:, g0 * H:(g0 + g) * H],
                              in_=og[:])
    nc.compile()
    return nc


def _get_nc(n_batches, g=32):
    key = (n_batches, g)
    if key not in _CACHE:
        _CACHE[key] = _build_nc(n_batches, g)
    return _CACHE[key]


def _dropout_keep_mask():
    """Reproduce the reference's threefry draw bit-exactly (CPU backend)."""
    if "keep" not in _CACHE:
        import jax
        cpu = jax.devices("cpu")[0]
        with jax.default_device(cpu):
            keep = jax.random.bernoulli(jax.random.key(42), 1.0 - DROP_P,
                                        (B, T, T))
            _CACHE["keep"] = np.asarray(keep)
    return _CACHE["keep"]


def _static_inputs(Wq, Wk, Wv):
    wqk = np.concatenate([Wq * 0.125, Wk], axis=1).astype(BF16_NP)
    wv = Wv.astype(BF16_NP)
    s_idx = np.arange(T)
    cneg1 = np.where(s_idx[:, None] <= s_idx[None, :], 0.0, -1e30).astype(
        BF16_NP)
    cneg = np.tile(cneg1, (1, 4))
    ident = np.eye(T, dtype=BF16_NP)
    return wqk, wv, cneg, ident


def _run(x, Wq, Wk, Wv, trace=False):
    from concourse.bass_utils import run_bass_kernel_spmd

    x = np.asarray(x, dtype=np.float32)
    Wq = np.asarray(Wq, dtype=np.float32)
    Wk = np.asarray(Wk, dtype=np.float32)
    Wv = np.asarray(Wv, dtype=np.float32)

    nc = _get_nc(B_PER_CORE)
    wqk, wv, cneg, ident = _static_inputs(Wq, Wk, Wv)

    keep = _dropout_keep_mask()
    import ml_dtypes as _mld
    dmask = np.where(keep, np.float32(1.25), np.float32(0.0)).astype(
        _mld.float8_e4m3)

    in_maps = []
    for i in range(N_CORES):
        sl = slice(i * B_PER_CORE, (i + 1) * B_PER_CORE)
        # x^T packed: [e, (b t)], bf16 (same rounding the DMA cast applied)
        xtp = np.ascontiguousarray(
            x[sl].transpose(2, 0, 1)).reshape(E, B_PER_CORE * T).astype(
                BF16_NP)
        # dropmask^T per batch: [s, (b t)]
        dtp = np.ascontiguousarray(
            dmask[sl].transpose(2, 0, 1)).reshape(T, B_PER_CORE * T)
        in_maps.append({
            "xtp": xtp, "dtp": dtp, "wqk": wqk, "wv": wv,
            "cneg": cneg, "ident": ident,
        })

    res = run_bass_kernel_spmd(nc, in_maps, list(range(N_CORES)), trace=trace)
    outs = []
    for i in range(N_CORES):
        o = res.results[i]["out"]  # [t, (b h)]
        outs.append(o.reshape(T, B_PER_CORE, H).transpose(1, 0, 2))
    full = np.ascontiguousarray(np.concatenate(outs, axis=0), dtype=np.float32)
    return full, res


def kernel(x, Wq, Wk, Wv):
    return _run(x, Wq, Wk, Wv, trace=False)[0]


# revision 24
# speedup vs baseline: 1.2988x; 1.0310x over previous
"""Trainium2 Bass kernel: causal single-head attention with attention-prob
dropout (train mode, fixed threefry key), data-parallel over batch on 8 cores.

Shapes (hardcoded): x [2048,128,256] f32, Wq/Wk/Wv [256,64] f32.
out [2048,128,64] f32.

Per-core design (256 batches/core):
  - host pre-packs x TRANSPOSED and bf16: xtp[e, b*T + t]; group DMA loads
    are contiguous per partition, and no on-device transpose is needed.
  - qkT = [Wq/8 | Wk].T @ x^T  -> [128, 128] psum (rows 0-63 qT, 64-127 kT)
  - v   = x @ Wv               -> [128, 64]
  - S^T in [s, t]: psum preloaded with causal -1e30 mask via an
    identity-stationary PE matmul, then += kT.T @ qT.
  - exp on ACT straight from PSUM (bf16 out); denominator d[t] via
    ones-matmul over s (partition reduction on PE); reciprocal on DVE.
  - numerator: PD = exp * dropmask({0,1.25}) (bf16), out = PD.T @ v,
    final scale by r[t] into the [t, h] output tile.
  - QUAD batching: psum tiles hold 4 batches per bank; exp / copies /
    dropout-mul are issued as single [128, 512]-ish ops over 4 batches.
"""

import os
import sys
import numpy as np

sys.path.insert(0, "/opt/trn_rl_repo")

import ml_dtypes

BF16_NP = ml_dtypes.bfloat16

B, T, E, H = 2048, 128, 256, 64
N_CORES = 8
B_PER_CORE = B // N_CORES
DROP_P = 0.2

_CACHE = {}


def _build_nc(n_batches, g=32):
    import concourse.bass as bass
    import concourse.mybir as mybir
    from concourse import bacc, tile
    from contextlib import ExitStack

    BF16 = mybir.dt.bfloat16
    F32 = mybir.dt.float32

    nc = bacc.Bacc()
    xt_p = nc.declare_dram_parameter("xtp", [E, n_batches * T], BF16,
                                     isOutput=False)
    FP8 = mybir.dt.float8e4
    dt_p = nc.declare_dram_parameter("dtp", [T, n_batches * T], FP8,
                                     isOutput=False)
    wqk_p = nc.declare_dram_parameter("wqk", [E, 128], BF16, isOutput=False)
    wv_p = nc.declare_dram_parameter("wv", [E, H], BF16, isOutput=False)
    cneg_p = nc.declare_dram_parameter("cneg", [T, 4 * T], BF16,
                                       isOutput=False)
    id_p = nc.declare_dram_parameter("ident", [T, T], BF16, isOutput=False)
    out_p = nc.declare_dram_parameter("out", [T, n_batches * H], F32,
                                      isOutput=True)

    n_groups = n_batches // g
    QUAD = 4
    n_quads = g // QUAD

    with tile.TileContext(nc) as tc, ExitStack() as ctx:
        const = ctx.enter_context(tc.tile_pool(name="const", bufs=1))
        wqk_sb = [const.tile([128, 128], BF16, tag=f"wqk{c}", name=f"wqk{c}")
                  for c in range(2)]
        wv_sb = [const.tile([128, H], BF16, tag=f"wv{c}", name=f"wv{c}")
                 for c in range(2)]
        cneg_sb = const.tile([T, 4 * T], BF16, tag="cneg", name="cneg")
        ident_sb = const.tile([T, T], BF16, tag="ident", name="ident")
        ones_sb = const.tile([128, 1], BF16, tag="ones", name="ones")
        for c in range(2):
            nc.sync.dma_start(out=wqk_sb[c][:],
                              in_=wqk_p[c * 128:(c + 1) * 128, :])
            nc.sync.dma_start(out=wv_sb[c][:],
                              in_=wv_p[c * 128:(c + 1) * 128, :])
        nc.sync.dma_start(out=cneg_sb[:], in_=cneg_p[:])
        nc.sync.dma_start(out=ident_sb[:], in_=id_p[:])
        nc.vector.memset(ones_sb[:], 1.0)

        xpool = ctx.enter_context(tc.tile_pool(name="xtg", bufs=2))
        dtpool = ctx.enter_context(tc.tile_pool(name="dtg", bufs=2))
        opool = ctx.enter_context(tc.tile_pool(name="og", bufs=2))
        work = ctx.enter_context(tc.tile_pool(name="work", bufs=6))
        ps_qk = ctx.enter_context(tc.tile_pool(name="ps_qk", bufs=2,
                                               space="PSUM"))
        ps_s = ctx.enter_context(tc.tile_pool(name="ps_s", bufs=2,
                                              space="PSUM"))
        ps_v = ctx.enter_context(tc.tile_pool(name="ps_v", bufs=2,
                                              space="PSUM"))
        ps_od = ctx.enter_context(tc.tile_pool(name="ps_od", bufs=2,
                                               space="PSUM"))

        EXP = mybir.ActivationFunctionType.Exp

        for grp in range(n_groups):
            g0 = grp * g
            xtg = [xpool.tile([128, g * T], BF16, tag=f"xtg{c}",
                              name=f"xtg{c}") for c in range(2)]
            dma_engs = [nc.sync, nc.scalar]
            for c in range(2):
                dma_engs[c].dma_start(
                    out=xtg[c][:],
                    in_=xt_p[c * 128:(c + 1) * 128, g0 * T:(g0 + g) * T])
            dtg = dtpool.tile([T, g * T], BF16, tag="dtg", name="dtg")
            nc.gpsimd.dma_start(out=dtg[:],
                                in_=dt_p[:, g0 * T:(g0 + g) * T])
            og = opool.tile([T, g * H], F32, tag="og", name="og")
            for q in range(n_quads):
                j0 = q * QUAD      # first batch (within group) of this quad

                # ---- PE: causal-mask preload into the S quad bank ----
                # ONE matmul covers the bank: start=True clears the whole
                # bank's has_written bits, so per-slice preloads would wipe
                # each other.
                p_s = ps_s.tile([128, QUAD * T], F32, tag="p_s", name="p_s")
                nc.tensor.matmul(p_s[:], ident_sb[:], cneg_sb[:],
                                 start=True, stop=False)

                # ---- PE: qkT for 4 batches, one weight load per chunk ----
                # only the first matmul touching the bank clears it
                p_qk = ps_qk.tile([128, QUAD * T], F32, tag="p_qk",
                                  name="p_qk")
                qsl = slice(j0 * T, (j0 + QUAD) * T)
                for c in range(2):
                    nc.tensor.matmul(p_qk[:], wqk_sb[c][:], xtg[c][:, qsl],
                                     start=(c == 0), stop=(c == 1))
                qt_sb = work.tile([64, QUAD * T], BF16, tag="qt_sb",
                                  name="qt_sb")
                nc.vector.tensor_copy(qt_sb[:], p_qk[0:64, :])
                # kT at base partition 0 (matmul requires matching bases);
                # ACT sits close to PSUM and has slack
                kt_sb = work.tile([64, QUAD * T], BF16, tag="kt_sb",
                                  name="kt_sb")
                nc.scalar.copy(kt_sb[:], p_qk[64:128, :])

                # ---- PE: v for 4 batches ----
                p_v = ps_v.tile([128, QUAD * H], F32, tag="p_v", name="p_v")
                for j in range(QUAD):
                    sl = slice((j0 + j) * T, (j0 + j + 1) * T)
                    for c in range(2):
                        nc.tensor.matmul(p_v[:, j * H:(j + 1) * H],
                                         xtg[c][:, sl], wv_sb[c][:],
                                         start=(c == 0 and j == 0),
                                         stop=(c == 1))
                v_sb = work.tile([128, QUAD * H], BF16, tag="v_sb",
                                 name="v_sb")
                nc.vector.tensor_copy(v_sb[:], p_v[:])

                # ---- PE: S^T += kT.T @ qT (accumulates onto mask) ----
                for j in range(QUAD):
                    nc.tensor.matmul(p_s[:, j * T:(j + 1) * T],
                                     kt_sb[:, j * T:(j + 1) * T],
                                     qt_sb[:, j * T:(j + 1) * T],
                                     start=False, stop=True)

                # ---- ACT: exp over the whole quad, psum -> sbuf bf16 ----
                e_sb = work.tile([128, QUAD * T], BF16, tag="e_sb",
                                 name="e_sb")
                nc.scalar.activation(e_sb[:], p_s[:], EXP)

                # ---- PE: denominators; DVE: reciprocal ----
                p_od = ps_od.tile([128, QUAD * (H + 1)], F32, tag="p_od",
                                  name="p_od")
                d_off = QUAD * H
                for j in range(QUAD):
                    nc.tensor.matmul(p_od[:, d_off + j:d_off + j + 1],
                                     e_sb[:, j * T:(j + 1) * T], ones_sb[:],
                                     start=(j == 0), stop=True)
                r_sb = work.tile([128, QUAD], F32, tag="r_sb", name="r_sb")
                nc.vector.reciprocal(r_sb[:], p_od[:, d_off:d_off + QUAD])

                # ---- DVE: dropout multiply over the quad ----
                pdm = work.tile([128, QUAD * T], BF16, tag="pdm", name="pdm")
                half = QUAD * T // 2
                nc.vector.tensor_mul(
                    pdm[:, 0:half], e_sb[:, 0:half],
                    dtg[:, j0 * T:j0 * T + half])
                nc.gpsimd.tensor_mul(
                    pdm[:, half:], e_sb[:, half:],
                    dtg[:, j0 * T + half:(j0 + QUAD) * T])

                # ---- PE: out numerator; DVE: per-batch r scale ----
                # start=False: the denominator slices live in the same bank
                # and must not be wiped; fresh elements overwrite via the
                # per-element has_written bits
                for j in range(QUAD):
                    nc.tensor.matmul(p_od[:, j * H:(j + 1) * H],
                                     pdm[:, j * T:(j + 1) * T],
                                     v_sb[:, j * H:(j + 1) * H],
                                     start=False, stop=True)
                nc.vector.tensor_mul(
                    og[:, j0 * H:(j0 + QUAD) * H].rearrange(
                        "p (f g) -> p f g", f=QUAD),
                    p_od[:, 0:QUAD * H].rearrange("p (f g) -> p f g", f=QUAD),
                    r_sb[:].broadcast_to([128, QUAD, H]))
            nc.gpsimd.dma_start(out=out_p[:, g0 * H:(g0 + g) * H],
                                in_=og[:])
    nc.compile()
    return nc


def _get_nc(n_batches, g=32):
    key = (n_batches, g)
    if key not in _CACHE:
        _CACHE[key] = _build_nc(n_batches, g)
    return _CACHE[key]


def _dropout_keep_mask():
    """Reproduce the reference's threefry draw bit-exactly (CPU backend)."""
    if "keep" not in _CACHE:
        import jax
        cpu = jax.devices("cpu")[0]
        with jax.default_device(cpu):
            keep = jax.random.bernoulli(jax.random.key(42), 1.0 - DROP_P,
                                        (B, T, T))
            _CACHE["keep"] = np.asarray(keep)
    return _CACHE["keep"]


def _static_inputs(Wq, Wk, Wv):
    wqk = np.concatenate([Wq * 0.125, Wk], axis=1).astype(BF16_NP)
    wv = Wv.astype(BF16_NP)
    s_idx = np.arange(T)
    cneg1 = np.where(s_idx[:, None] <= s_idx[None, :], 0.0, -1e30).astype(
        BF16_NP)
    cneg = np.tile(cneg1, (1, 4))
    ident = np.eye(T, dtype=BF16_NP)
    return wqk, wv, cneg, ident


def _run(x, Wq, Wk, Wv, trace=False):
    from concourse.bass_utils import run_bass_kernel_spmd

    x = np.asarray(x, dtype=np.float32)
    Wq = np.asarray(Wq, dtype=np.float32)
    Wk = np.asarray(Wk, dtype=np.float32)
    Wv = np.asarray(Wv, dtype=np.float32)

    nc = _get_nc(B_PER_CORE)
    wqk, wv, cneg, ident = _static_inputs(Wq, Wk, Wv)

    keep = _dropout_keep_mask()
    import ml_dtypes as _mld
    dmask = np.where(keep, np.float32(1.25), np.float32(0.0)).astype(
        _mld.float8_e4m3)

    in_maps = []
    for i in range(N_CORES):
        sl = slice(i * B_PER_CORE, (i + 1) * B_PER_CORE)
        # x^T packed: [e, (b t)], bf16 (same rounding the DMA cast applied)
        xtp = np.ascontiguousarray(
            x[sl].transpose(2, 0, 1)).reshape(E, B_PER_CORE * T).astype(
                BF16_NP)
        # dropmask^T per batch: [s, (b t)]
        dtp = np.ascontiguousarray(
            dmask[sl].transpose(2, 0, 1)).reshape(T, B_PER_CORE * T)
        in_maps.append({
            "xtp": xtp, "dtp": dtp, "wqk": wqk, "wv": wv,
            "cneg": cneg, "ident": ident,
        })

    res = run_bass_kernel_spmd(nc, in_maps, list(range(N_CORES)), trace=trace)
    outs = []
    for i in range(N_CORES):
        o = res.results[i]["out"]  # [t, (b h)]
        outs.append(o.reshape(T, B_PER_CORE, H).transpose(1, 0, 2))
    full = np.ascontiguousarray(np.concatenate(outs, axis=0), dtype=np.float32)
    return full, res


def kernel(x, Wq, Wk, Wv):
    return _run(x, Wq, Wk, Wv, trace=False)[0]


# revision 25
# speedup vs baseline: 1.4259x; 1.0978x over previous
"""Trainium2 Bass kernel: causal single-head attention with attention-prob
dropout (train mode, fixed threefry key), data-parallel over batch on 8 cores.

Shapes (hardcoded): x [2048,128,256] f32, Wq/Wk/Wv [256,64] f32.
out [2048,128,64] f32.

Per-core design (256 batches/core):
  - host pre-packs x TRANSPOSED and bf16: xtp[e, b*T + t]; group DMA loads
    are contiguous per partition, and no on-device transpose is needed.
  - qkT = [Wq/8 | Wk].T @ x^T  -> [128, 128] psum (rows 0-63 qT, 64-127 kT)
  - v   = x @ Wv               -> [128, 64]
  - S^T in [s, t]: psum preloaded with causal -1e30 mask via an
    identity-stationary PE matmul, then += kT.T @ qT.
  - exp on ACT straight from PSUM (bf16 out); denominator d[t] via
    ones-matmul over s (partition reduction on PE); reciprocal on DVE.
  - numerator: PD = exp * dropmask({0,1.25}) (bf16), out = PD.T @ v,
    final scale by r[t] into the [t, h] output tile.
  - QUAD batching: psum tiles hold 4 batches per bank; exp / copies /
    dropout-mul are issued as single [128, 512]-ish ops over 4 batches.
"""

import os
import sys
import numpy as np

sys.path.insert(0, "/opt/trn_rl_repo")

import ml_dtypes

BF16_NP = ml_dtypes.bfloat16

B, T, E, H = 2048, 128, 256, 64
N_CORES = 8
B_PER_CORE = B // N_CORES
DROP_P = 0.2

_CACHE = {}


def _build_nc(n_batches, g=32):
    import concourse.bass as bass
    import concourse.mybir as mybir
    from concourse import bacc, tile
    from contextlib import ExitStack

    BF16 = mybir.dt.bfloat16
    F32 = mybir.dt.float32

    nc = bacc.Bacc()
    xt_p = nc.declare_dram_parameter("xtp", [E, n_batches * T], BF16,
                                     isOutput=False)
    FP8 = mybir.dt.float8e4
    dt_p = nc.declare_dram_parameter("dtp", [T, n_batches * T], FP8,
                                     isOutput=False)
    wqk_p = nc.declare_dram_parameter("wqk", [E, 128], BF16, isOutput=False)
    wv_p = nc.declare_dram_parameter("wv", [E, H], BF16, isOutput=False)
    cneg_p = nc.declare_dram_parameter("cneg", [T, 4 * T], BF16,
                                       isOutput=False)
    id_p = nc.declare_dram_parameter("ident", [T, T], BF16, isOutput=False)
    out_p = nc.declare_dram_parameter("out", [T, n_batches * H], F32,
                                      isOutput=True)

    n_groups = n_batches // g
    QUAD = 4
    n_quads = g // QUAD

    with tile.TileContext(nc) as tc, ExitStack() as ctx:
        const = ctx.enter_context(tc.tile_pool(name="const", bufs=1))
        wqk_sb = [const.tile([128, 128], BF16, tag=f"wqk{c}", name=f"wqk{c}")
                  for c in range(2)]
        wv_sb = [const.tile([128, H], BF16, tag=f"wv{c}", name=f"wv{c}")
                 for c in range(2)]
        cneg_sb = const.tile([T, 4 * T], BF16, tag="cneg", name="cneg")
        ident_sb = const.tile([T, T], BF16, tag="ident", name="ident")
        ones_sb = const.tile([128, 1], BF16, tag="ones", name="ones")
        for c in range(2):
            nc.sync.dma_start(out=wqk_sb[c][:],
                              in_=wqk_p[c * 128:(c + 1) * 128, :])
            nc.sync.dma_start(out=wv_sb[c][:],
                              in_=wv_p[c * 128:(c + 1) * 128, :])
        nc.sync.dma_start(out=cneg_sb[:], in_=cneg_p[:])
        nc.sync.dma_start(out=ident_sb[:], in_=id_p[:])
        nc.vector.memset(ones_sb[:], 1.0)

        xpool = ctx.enter_context(tc.tile_pool(name="xtg", bufs=2))
        dtpool = ctx.enter_context(tc.tile_pool(name="dtg", bufs=2))
        opool = ctx.enter_context(tc.tile_pool(name="og", bufs=2))
        work = ctx.enter_context(tc.tile_pool(name="work", bufs=6))
        ps_qk = ctx.enter_context(tc.tile_pool(name="ps_qk", bufs=2,
                                               space="PSUM"))
        ps_s = ctx.enter_context(tc.tile_pool(name="ps_s", bufs=2,
                                              space="PSUM"))
        ps_v = ctx.enter_context(tc.tile_pool(name="ps_v", bufs=2,
                                              space="PSUM"))
        ps_od = ctx.enter_context(tc.tile_pool(name="ps_od", bufs=2,
                                               space="PSUM"))

        EXP = mybir.ActivationFunctionType.Exp

        for grp in range(n_groups):
            g0 = grp * g
            xtg = [xpool.tile([128, g * T], BF16, tag=f"xtg{c}",
                              name=f"xtg{c}") for c in range(2)]
            dma_engs = [nc.sync, nc.scalar]
            for c in range(2):
                dma_engs[c].dma_start(
                    out=xtg[c][:],
                    in_=xt_p[c * 128:(c + 1) * 128, g0 * T:(g0 + g) * T])
            dtg = dtpool.tile([T, g * T], BF16, tag="dtg", name="dtg")
            nc.gpsimd.dma_start(out=dtg[:],
                                in_=dt_p[:, g0 * T:(g0 + g) * T])
            og = opool.tile([T, g * H], F32, tag="og", name="og")
            for q in range(n_quads):
                j0 = q * QUAD      # first batch (within group) of this quad

                # ---- PE: causal-mask preload into the S quad bank ----
                # ONE matmul covers the bank: start=True clears the whole
                # bank's has_written bits, so per-slice preloads would wipe
                # each other.
                p_s = ps_s.tile([128, QUAD * T], F32, tag="p_s", name="p_s")
                nc.tensor.matmul(p_s[:], ident_sb[:], cneg_sb[:],
                                 start=True, stop=False)

                # ---- PE: qkT for 4 batches, one weight load per chunk ----
                # only the first matmul touching the bank clears it
                p_qk = ps_qk.tile([128, QUAD * T], F32, tag="p_qk",
                                  name="p_qk")
                qsl = slice(j0 * T, (j0 + QUAD) * T)
                for c in range(2):
                    nc.tensor.matmul(p_qk[:], wqk_sb[c][:], xtg[c][:, qsl],
                                     start=(c == 0), stop=(c == 1))
                qt_sb = work.tile([64, QUAD * T], BF16, tag="qt_sb",
                                  name="qt_sb")
                nc.vector.tensor_copy(qt_sb[:], p_qk[0:64, :])
                # kT at base partition 0 (matmul requires matching bases);
                # ACT sits close to PSUM and has slack
                kt_sb = work.tile([64, QUAD * T], BF16, tag="kt_sb",
                                  name="kt_sb")
                nc.scalar.copy(kt_sb[:], p_qk[64:128, :])

                # ---- PE: v for 4 batches ----
                p_v = ps_v.tile([128, QUAD * H], F32, tag="p_v", name="p_v")
                for j in range(QUAD):
                    sl = slice((j0 + j) * T, (j0 + j + 1) * T)
                    for c in range(2):
                        nc.tensor.matmul(p_v[:, j * H:(j + 1) * H],
                                         xtg[c][:, sl], wv_sb[c][:],
                                         start=(c == 0 and j == 0),
                                         stop=(c == 1))
                v_sb = work.tile([128, QUAD * H], BF16, tag="v_sb",
                                 name="v_sb")
                nc.vector.tensor_copy(v_sb[:], p_v[:])

                # ---- PE: S^T += kT.T @ qT (accumulates onto mask) ----
                for j in range(QUAD):
                    nc.tensor.matmul(p_s[:, j * T:(j + 1) * T],
                                     kt_sb[:, j * T:(j + 1) * T],
                                     qt_sb[:, j * T:(j + 1) * T],
                                     start=False, stop=True)

                # ---- ACT: exp over the whole quad, psum -> sbuf bf16 ----
                e_sb = work.tile([128, QUAD * T], BF16, tag="e_sb",
                                 name="e_sb")
                nc.scalar.activation(e_sb[:], p_s[:], EXP)

                # ---- PE: denominators; DVE: reciprocal ----
                p_od = ps_od.tile([128, QUAD * (H + 1)], F32, tag="p_od",
                                  name="p_od")
                d_off = QUAD * H
                for j in range(QUAD):
                    nc.tensor.matmul(p_od[:, d_off + j:d_off + j + 1],
                                     e_sb[:, j * T:(j + 1) * T], ones_sb[:],
                                     start=(j == 0), stop=True)
                r_sb = work.tile([128, QUAD], F32, tag="r_sb", name="r_sb")
                nc.vector.reciprocal(r_sb[:], p_od[:, d_off:d_off + QUAD])

                # ---- DVE: dropout multiply over the quad ----
                pdm = work.tile([128, QUAD * T], BF16, tag="pdm", name="pdm")
                nc.gpsimd.tensor_mul(
                    pdm[:], e_sb[:], dtg[:, j0 * T:(j0 + QUAD) * T])

                # ---- PE: out numerator; DVE: per-batch r scale ----
                # start=False: the denominator slices live in the same bank
                # and must not be wiped; fresh elements overwrite via the
                # per-element has_written bits
                for j in range(QUAD):
                    nc.tensor.matmul(p_od[:, j * H:(j + 1) * H],
                                     pdm[:, j * T:(j + 1) * T],
                                     v_sb[:, j * H:(j + 1) * H],
                                     start=False, stop=True)
                nc.vector.tensor_mul(
                    og[:, j0 * H:(j0 + QUAD) * H].rearrange(
                        "p (f g) -> p f g", f=QUAD),
                    p_od[:, 0:QUAD * H].rearrange("p (f g) -> p f g", f=QUAD),
                    r_sb[:].broadcast_to([128, QUAD, H]))
            nc.gpsimd.dma_start(out=out_p[:, g0 * H:(g0 + g) * H],
                                in_=og[:])
    nc.compile()
    return nc


def _get_nc(n_batches, g=32):
    key = (n_batches, g)
    if key not in _CACHE:
        _CACHE[key] = _build_nc(n_batches, g)
    return _CACHE[key]


def _dropout_keep_mask():
    """Reproduce the reference's threefry draw bit-exactly (CPU backend)."""
    if "keep" not in _CACHE:
        import jax
        cpu = jax.devices("cpu")[0]
        with jax.default_device(cpu):
            keep = jax.random.bernoulli(jax.random.key(42), 1.0 - DROP_P,
                                        (B, T, T))
            _CACHE["keep"] = np.asarray(keep)
    return _CACHE["keep"]


def _static_inputs(Wq, Wk, Wv):
    wqk = np.concatenate([Wq * 0.125, Wk], axis=1).astype(BF16_NP)
    wv = Wv.astype(BF16_NP)
    s_idx = np.arange(T)
    cneg1 = np.where(s_idx[:, None] <= s_idx[None, :], 0.0, -1e30).astype(
        BF16_NP)
    cneg = np.tile(cneg1, (1, 4))
    ident = np.eye(T, dtype=BF16_NP)
    return wqk, wv, cneg, ident


def _run(x, Wq, Wk, Wv, trace=False):
    from concourse.bass_utils import run_bass_kernel_spmd

    x = np.asarray(x, dtype=np.float32)
    Wq = np.asarray(Wq, dtype=np.float32)
    Wk = np.asarray(Wk, dtype=np.float32)
    Wv = np.asarray(Wv, dtype=np.float32)

    nc = _get_nc(B_PER_CORE)
    wqk, wv, cneg, ident = _static_inputs(Wq, Wk, Wv)

    keep = _dropout_keep_mask()
    import ml_dtypes as _mld
    dmask = np.where(keep, np.float32(1.25), np.float32(0.0)).astype(
        _mld.float8_e4m3)

    in_maps = []
    for i in range(N_CORES):
        sl = slice(i * B_PER_CORE, (i + 1) * B_PER_CORE)
        # x^T packed: [e, (b t)], bf16 (same rounding the DMA cast applied)
        xtp = np.ascontiguousarray(
            x[sl].transpose(2, 0, 1)).reshape(E, B_PER_CORE * T).astype(
                BF16_NP)
        # dropmask^T per batch: [s, (b t)]
        dtp = np.ascontiguousarray(
            dmask[sl].transpose(2, 0, 1)).reshape(T, B_PER_CORE * T)
        in_maps.append({
            "xtp": xtp, "dtp": dtp, "wqk": wqk, "wv": wv,
            "cneg": cneg, "ident": ident,
        })

    res = run_bass_kernel_spmd(nc, in_maps, list(range(N_CORES)), trace=trace)
    outs = []
    for i in range(N_CORES):
        o = res.results[i]["out"]  # [t, (b h)]
        outs.append(o.reshape(T, B_PER_CORE, H).transpose(1, 0, 2))
    full = np.ascontiguousarray(np.concatenate(outs, axis=0), dtype=np.float32)
    return full, res


def kernel(x, Wq, Wk, Wv):
    return _run(x, Wq, Wk, Wv, trace=False)[0]


# revision 38
# speedup vs baseline: 1.5213x; 1.0670x over previous
"""Trainium2 Bass kernel: causal single-head attention with attention-prob
dropout (train mode, fixed threefry key), data-parallel over batch on 8 cores.

Shapes (hardcoded): x [2048,128,256] f32, Wq/Wk/Wv [256,64] f32.
out [2048,128,64] f32.

Per-core design (256 batches/core):
  - host pre-packs x TRANSPOSED and bf16: xtp[e, b*T + t]; group DMA loads
    are contiguous per partition, and no on-device transpose is needed.
    The dropout mask ships as fp8e4 ({0, 1.25} both exact) and is consumed
    directly by the mixed-dtype multiply.
  - qkT = [Wq/8 | Wk].T @ x^T  -> [128, 128] psum (rows 0-63 qT, 64-127 kT)
  - v   = x @ Wv               -> [128, 64]
  - S^T in [s, t]: psum preloaded with causal -1e30 mask via an
    identity-stationary PE matmul, then += kT.T @ qT.
  - exp on ACT straight from PSUM (bf16 out); denominator d[t] via
    ones-matmul over s (partition reduction on PE); reciprocal on DVE.
  - numerator: PD = exp * dropmask({0,1.25}) (bf16), out = PD.T @ v,
    final scale by r[t] into the [t, h] output tile.
  - QUAD batching: psum tiles hold 4 batches per bank; exp / copies /
    dropout-mul are issued as single [128, 512]-ish ops over 4 batches.
  - group sizes ramp 8..64..8 to shorten pipeline fill/drain; streams are
    spread across the sync/scalar HWDGE rings and the gpsimd SWDGE ring.
"""

import os
import sys
import numpy as np

sys.path.insert(0, "/opt/trn_rl_repo")

import ml_dtypes

BF16_NP = ml_dtypes.bfloat16

B, T, E, H = 2048, 128, 256, 64
N_CORES = 8
B_PER_CORE = B // N_CORES
DROP_P = 0.2

_CACHE = {}


def _build_nc(n_batches, g=64):
    import concourse.bass as bass
    import concourse.mybir as mybir
    from concourse import bacc, tile
    from contextlib import ExitStack

    BF16 = mybir.dt.bfloat16
    F32 = mybir.dt.float32

    nc = bacc.Bacc()
    xt_p = nc.declare_dram_parameter("xtp", [E, n_batches * T], BF16,
                                     isOutput=False)
    FP8 = mybir.dt.float8e4
    dt_p = nc.declare_dram_parameter("dtp", [T, n_batches * T], FP8,
                                     isOutput=False)
    wqk_p = nc.declare_dram_parameter("wqk", [E, 128], BF16, isOutput=False)
    wv_p = nc.declare_dram_parameter("wv", [E, H], BF16, isOutput=False)
    cneg_p = nc.declare_dram_parameter("cneg", [T, 4 * T], BF16,
                                       isOutput=False)
    id_p = nc.declare_dram_parameter("ident", [T, T], BF16, isOutput=False)
    out_p = nc.declare_dram_parameter("out", [T, n_batches * H], F32,
                                      isOutput=True)

    # ramp-up/ramp-down group sizes: small edge groups cut the pipeline
    # fill/drain stalls; big middle groups keep DMA transfers large
    if n_batches >= 256:
        sizes = [8, 16, 32, 64, 64, 48, 16, 8]
        assert sum(sizes) == n_batches, sizes
    else:
        sizes = [min(g, n_batches)] * (n_batches // min(g, n_batches))
    QUAD = 4

    with tile.TileContext(nc) as tc, ExitStack() as ctx:
        const = ctx.enter_context(tc.tile_pool(name="const", bufs=1))
        wqk_sb = [const.tile([128, 128], BF16, tag=f"wqk{c}", name=f"wqk{c}")
                  for c in range(2)]
        wv_sb = [const.tile([128, H], BF16, tag=f"wv{c}", name=f"wv{c}")
                 for c in range(2)]
        cneg_sb = const.tile([T, 4 * T], BF16, tag="cneg", name="cneg")
        ident_sb = const.tile([T, T], BF16, tag="ident", name="ident")
        ones_sb = const.tile([128, 1], BF16, tag="ones", name="ones")
        for c in range(2):
            nc.sync.dma_start(out=wqk_sb[c][:],
                              in_=wqk_p[c * 128:(c + 1) * 128, :])
            nc.sync.dma_start(out=wv_sb[c][:],
                              in_=wv_p[c * 128:(c + 1) * 128, :])
        nc.sync.dma_start(out=cneg_sb[:], in_=cneg_p[:])
        nc.sync.dma_start(out=ident_sb[:], in_=id_p[:])
        nc.vector.memset(ones_sb[:], 1.0)

        xpool = ctx.enter_context(tc.tile_pool(name="xtg", bufs=2))
        dtpool = ctx.enter_context(tc.tile_pool(name="dtg", bufs=2))
        opool = ctx.enter_context(tc.tile_pool(name="og", bufs=2))
        work = ctx.enter_context(tc.tile_pool(name="work", bufs=6))
        ps_qk = ctx.enter_context(tc.tile_pool(name="ps_qk", bufs=2,
                                               space="PSUM"))
        ps_s = ctx.enter_context(tc.tile_pool(name="ps_s", bufs=2,
                                              space="PSUM"))
        ps_v = ctx.enter_context(tc.tile_pool(name="ps_v", bufs=2,
                                              space="PSUM"))
        ps_od = ctx.enter_context(tc.tile_pool(name="ps_od", bufs=2,
                                               space="PSUM"))

        EXP = mybir.ActivationFunctionType.Exp

        g0 = 0
        for grp, g in enumerate(sizes):
            n_quads = g // QUAD
            xtg = [xpool.tile([128, g * T], BF16, tag=f"xtg{c}",
                              name=f"xtg{c}") for c in range(2)]
            # split loads so early quads unblock before the full group
            dma_engs = [nc.sync, nc.scalar]
            n_split = g // 16 if g % 16 == 0 else 1
            sp = g * T // n_split
            assert sp * n_split == g * T
            for c in range(2):
                for k in range(n_split):
                    dma_engs[c].dma_start(
                        out=xtg[c][:, k * sp:(k + 1) * sp],
                        in_=xt_p[c * 128:(c + 1) * 128,
                                 g0 * T + k * sp:g0 * T + (k + 1) * sp])
            dtg = dtpool.tile([T, g * T], FP8, tag="dtg", name="dtg")
            n_dsp = g // 32 if g % 32 == 0 else 1
            dsp = g * T // n_dsp
            assert dsp * n_dsp == g * T
            for k in range(n_dsp):
                nc.gpsimd.dma_start(
                    out=dtg[:, k * dsp:(k + 1) * dsp],
                    in_=dt_p[:, g0 * T + k * dsp:g0 * T + (k + 1) * dsp])
            og = opool.tile([T, g * H], F32, tag="og", name="og")
            for q in range(n_quads):
                j0 = q * QUAD      # first batch (within group) of this quad

                # ---- PE: causal-mask preload into the S quad bank ----
                # ONE matmul covers the bank: start=True clears the whole
                # bank's has_written bits, so per-slice preloads would wipe
                # each other.
                p_s = ps_s.tile([128, QUAD * T], F32, tag="p_s", name="p_s")
                nc.tensor.matmul(p_s[:], ident_sb[:], cneg_sb[:],
                                 start=True, stop=False)

                # ---- PE: qkT for 4 batches, one weight load per chunk ----
                # only the first matmul touching the bank clears it
                p_qk = ps_qk.tile([128, QUAD * T], F32, tag="p_qk",
                                  name="p_qk")
                qsl = slice(j0 * T, (j0 + QUAD) * T)
                for c in range(2):
                    nc.tensor.matmul(p_qk[:], wqk_sb[c][:], xtg[c][:, qsl],
                                     start=(c == 0), stop=(c == 1))
                qt_sb = work.tile([64, QUAD * T], BF16, tag="qt_sb",
                                  name="qt_sb")
                nc.vector.tensor_copy(qt_sb[:], p_qk[0:64, :])
                # kT at base partition 0 (matmul requires matching bases);
                # ACT sits close to PSUM and has slack
                kt_sb = work.tile([64, QUAD * T], BF16, tag="kt_sb",
                                  name="kt_sb")
                nc.scalar.copy(kt_sb[:], p_qk[64:128, :])

                # ---- PE: v for 4 batches ----
                p_v = ps_v.tile([128, QUAD * H], F32, tag="p_v", name="p_v")
                for j in range(QUAD):
                    sl = slice((j0 + j) * T, (j0 + j + 1) * T)
                    for c in range(2):
                        nc.tensor.matmul(p_v[:, j * H:(j + 1) * H],
                                         xtg[c][:, sl], wv_sb[c][:],
                                         start=(c == 0 and j == 0),
                                         stop=(c == 1))
                v_sb = work.tile([128, QUAD * H], BF16, tag="v_sb",
                                 name="v_sb")
                nc.vector.tensor_copy(v_sb[:], p_v[:])

                # ---- PE: S^T += kT.T @ qT (accumulates onto mask) ----
                for j in range(QUAD):
                    nc.tensor.matmul(p_s[:, j * T:(j + 1) * T],
                                     kt_sb[:, j * T:(j + 1) * T],
                                     qt_sb[:, j * T:(j + 1) * T],
                                     start=False, stop=True)

                # ---- ACT: exp over the whole quad, psum -> sbuf bf16 ----
                e_sb = work.tile([128, QUAD * T], BF16, tag="e_sb",
                                 name="e_sb")
                nc.scalar.activation(e_sb[:], p_s[:], EXP)

                # ---- PE: denominators; DVE: reciprocal ----
                p_od = ps_od.tile([128, QUAD * (H + 1)], F32, tag="p_od",
                                  name="p_od")
                d_off = QUAD * H
                for j in range(QUAD):
                    nc.tensor.matmul(p_od[:, d_off + j:d_off + j + 1],
                                     e_sb[:, j * T:(j + 1) * T], ones_sb[:],
                                     start=(j == 0), stop=True)
                r_sb = work.tile([128, QUAD], F32, tag="r_sb", name="r_sb")
                nc.vector.reciprocal(r_sb[:], p_od[:, d_off:d_off + QUAD])

                # ---- GpSimd: dropout multiply over the quad ----
                pdm = work.tile([128, QUAD * T], BF16, tag="pdm", name="pdm")
                nc.gpsimd.tensor_mul(
                    pdm[:], e_sb[:], dtg[:, j0 * T:(j0 + QUAD) * T])

                # ---- PE: out numerator; DVE: fused r scale ----
                # start=False: the denominator slices live in the same bank
                # and must not be wiped; fresh elements overwrite via the
                # per-element has_written bits
                for j in range(QUAD):
                    nc.tensor.matmul(p_od[:, j * H:(j + 1) * H],
                                     pdm[:, j * T:(j + 1) * T],
                                     v_sb[:, j * H:(j + 1) * H],
                                     start=False, stop=True)
                nc.vector.tensor_mul(
                    og[:, j0 * H:(j0 + QUAD) * H].rearrange(
                        "p (f g) -> p f g", f=QUAD),
                    p_od[:, 0:QUAD * H].rearrange("p (f g) -> p f g", f=QUAD),
                    r_sb[:].broadcast_to([128, QUAD, H]))
            n_osp = g // 32 if g % 32 == 0 else 1
            oh = g * H // n_osp
            assert oh * n_osp == g * H
            for k in range(n_osp):
                nc.gpsimd.dma_start(
                    out=out_p[:, g0 * H + k * oh:g0 * H + (k + 1) * oh],
                    in_=og[:, k * oh:(k + 1) * oh])
            g0 += g
    nc.compile()
    return nc


def _get_nc(n_batches, g=64):
    key = (n_batches, g)
    if key not in _CACHE:
        _CACHE[key] = _build_nc(n_batches, g)
    return _CACHE[key]


def _dropout_keep_mask():
    """Reproduce the reference's threefry draw bit-exactly (CPU backend)."""
    if "keep" not in _CACHE:
        import jax
        cpu = jax.devices("cpu")[0]
        with jax.default_device(cpu):
            keep = jax.random.bernoulli(jax.random.key(42), 1.0 - DROP_P,
                                        (B, T, T))
            _CACHE["keep"] = np.asarray(keep)
    return _CACHE["keep"]


def _static_inputs(Wq, Wk, Wv):
    wqk = np.concatenate([Wq * 0.125, Wk], axis=1).astype(BF16_NP)
    wv = Wv.astype(BF16_NP)
    s_idx = np.arange(T)
    cneg1 = np.where(s_idx[:, None] <= s_idx[None, :], 0.0, -1e30).astype(
        BF16_NP)
    cneg = np.tile(cneg1, (1, 4))
    ident = np.eye(T, dtype=BF16_NP)
    return wqk, wv, cneg, ident


def _run(x, Wq, Wk, Wv, trace=False):
    from concourse.bass_utils import run_bass_kernel_spmd

    x = np.asarray(x, dtype=np.float32)
    Wq = np.asarray(Wq, dtype=np.float32)
    Wk = np.asarray(Wk, dtype=np.float32)
    Wv = np.asarray(Wv, dtype=np.float32)

    nc = _get_nc(B_PER_CORE)
    wqk, wv, cneg, ident = _static_inputs(Wq, Wk, Wv)

    keep = _dropout_keep_mask()
    import ml_dtypes as _mld
    dmask = np.where(keep, np.float32(1.25), np.float32(0.0)).astype(
        _mld.float8_e4m3)

    in_maps = []
    for i in range(N_CORES):
        sl = slice(i * B_PER_CORE, (i + 1) * B_PER_CORE)
        # x^T packed: [e, (b t)], bf16 (same rounding the DMA cast applied)
        xtp = np.ascontiguousarray(
            x[sl].transpose(2, 0, 1)).reshape(E, B_PER_CORE * T).astype(
                BF16_NP)
        # dropmask^T per batch: [s, (b t)]
        dtp = np.ascontiguousarray(
            dmask[sl].transpose(2, 0, 1)).reshape(T, B_PER_CORE * T)
        in_maps.append({
            "xtp": xtp, "dtp": dtp, "wqk": wqk, "wv": wv,
            "cneg": cneg, "ident": ident,
        })

    res = run_bass_kernel_spmd(nc, in_maps, list(range(N_CORES)), trace=trace)
    outs = []
    for i in range(N_CORES):
        o = res.results[i]["out"]  # [t, (b h)]
        outs.append(o.reshape(T, B_PER_CORE, H).transpose(1, 0, 2))
    full = np.ascontiguousarray(np.concatenate(outs, axis=0), dtype=np.float32)
    return full, res


def kernel(x, Wq, Wk, Wv):
    return _run(x, Wq, Wk, Wv, trace=False)[0]

